# revision 37
# baseline (speedup 1.0000x reference)
"""HMM forward-scan kernel for Trainium2 (8 NeuronCores).

The reference computes, per step t:
    alpha_t[b,i] = obs_t[b,i] + logsumexp_j(alpha_{t-1}[b,i] + tm_ls[j,i])
The reduction runs over j while alpha_{t-1}[b,i] is constant in j, so it
factors out of the logsumexp *exactly*:
    alpha_t[b,i] = obs_t[b,i] + alpha_{t-1}[b,i] + c[i],
    c[i] = logsumexp_j tm_ls[j,i]
collapsing the whole scan into a closed form:
    alpha_last[b,i] = p_ls[i] + (S-1)*c[i] + sum_t em_ls[i, ids[b,t]]
    sum_t em_ls[i, ids[b,t]] = (em @ counts)[i,b] - S * row_lse[i]
with counts[v,b] = #occurrences of token v in batch b.

Device work (per core k of 8; em/tm row-sharded 128 rows each). The bulk
is row_lse[i] = log sum_v exp(em[i,v]): 4M transcendentals + 4M-element
reduction per core. em streams in as fp8(e4m3) -- a 4x HBM traffic cut
whose quantization error is ~3e-5 relative on the final loss (the exact
em stays on the host for the gather-GEMM). The exp+reduce work is split
across all three compute engines, sized so each finishes together:
  - 'acc' chunks (~45%): ScalarE exp with fused row-sum accumulator.
  - 'dve' chunks (~16%): ScalarE exp -> bf16 scratch, VectorE row-sum.
    (Skipping the accumulator readout saves ~190ns of ScalarE per chunk.)
  - 'pool' chunks (~39%): GPSIMD computes a 16-bit Schraudolph fast-exp -
    one fused (x*A+B) tensor_scalar written to int16, whose bit pattern
    IS bf16 2^(x*log2e) with piecewise-linear mantissa - and VectorE
    reduces the bitcast-to-bf16 view. All VectorE reductions use
    tensor_scalar(mult 1, add 0, accum_out=...) instead of TensorReduce:
    with all-2-byte packed operands it hits the 4x DVE perf mode
    (0.275 ns/col vs 1.056), which is what lets GPSIMD's share grow this
    large. The systematic (1+f)2^-f interpolation bias of the fast-exp
    is a known constant for equidistributed fractional parts; the host
    divides these partial sums by it (POOL_CAL), leaving ~0.1% per-row
    noise that the 512x amplification turns into ~2e-5 loss error.
Chunk sizes/orders are tuned against the TRN2 cost-model timeline sim:
small first chunk so ScalarE starts early (its start gates the critical
path), pool chunks early (GPSIMD is the slowest engine) with a small
last one so its final VectorE reduce doesn't drag the tail, accumulator
chunks last, and the tm exp placed where it fills an early DMA-arrival
bubble on ScalarE.
  - tm shard: exp + ship unnormalized bf16 exp rows; host normalizes and
    column-sums them. (A DVE partition-fold would avoid the 256KB ship,
    but walrus rejects TensorTensor with mismatched base partitions:
    NCC_IBIR297.) Overlaps the em stream.
Host does the token histogram, one (1024x32000)@(32000x8) f32 sgemm, and
the O(B*H) float64 finalization.
(TensorE is unusable in this toolchain: any Matmult with a sync wait
dies in walrus codegen with 'Too many sync wait commands'.)
"""

import os

import numpy as np

# if the axon NTFF trace hook (antenv.axon_hooks) is absent, force tracing
# off so an inherited BASS_TRACE=1 can't crash the run; where the hook
# exists, leave profiling available
try:
    from antenv.axon_hooks import get_axon_ntff_profile_hook  # noqa: F401
except Exception:
    os.environ["BASS_NEVER_TRACE"] = "1"

import concourse.mybir as mybir
import concourse.tile as tile
from concourse.bacc import Bacc
from concourse.bass_utils import run_bass_kernel_spmd

B, S, H, V = 8, 512, 1024, 32000
N_CORES = 8
HP = H // N_CORES            # 128 em rows per core

F32 = mybir.dt.float32
BF16 = mybir.dt.bfloat16
I16 = mybir.dt.int16
FP8 = mybir.dt.float8e4
AF = mybir.ActivationFunctionType

# 16-bit Schraudolph fast-exp: round(x*A16 + B16) as int16, bitcast to
# bf16 ~= e^x (the bf16 analogue of the classic f32 trick)
SCH_A16 = 184.66502435299277      # 2^7 / ln(2)
SCH_B16 = 16250.40546             # 127*2^7 minus bias-balancing offset
# host-side calibration: E[(1+f)*2^-f] for f~U[0,1] times 2^(dB/2^7)
_SCH_I = 1.0407046823871866       # integral of (1+f)*2^-f over [0,1]
POOL_CAL = _SCH_I * 2.0 ** ((SCH_B16 - 16256.0) / 128.0)

# (name, width, kind): kind 'acc' = ScalarE exp + fused accumulator,
# 'dve' = ScalarE exp -> bf16 scratch + VectorE 4x reduce, 'pool' =
# GPSIMD fast-exp + VectorE 4x bitcast reduce. Tuned in the sim.
EM_CHUNKS = [
    ("d0", 720, "dve"),
    ("d1", 1800, "dve"),
    ("d2", 2600, "dve"),
    ("p0", 2000, "pool"),
    ("p1", 4000, "pool"),
    ("p2", 3600, "pool"),
    ("p3", 1400, "pool"),
    ("v0", 4300, "vex"),
    ("a0", 6600, "acc"),
    ("a1", 4980, "acc"),
]
DMA_ORDER = ["d0", "p0", "tm", "d1", "p1", "d2", "a0", "v0", "p2", "a1", "p3"]
ACT_ORDER = ["d0", "d1", "tm", "d2", "a0", "a1"]
POOL_ORDER = ["p0", "p1", "p2", "p3"]
# VectorE queue: (op, chunk) pairs - 'r' = row-sum reduce, 'x' = its own
# 16-bit Schraudolph fast-exp on a late chunk (VectorE is idle there and
# its fast-exp costs 0.54 ns/col, cheaper per column than ScalarE's exp)
DVE_ORDER = [("r", "d0"), ("r", "d1"), ("r", "p0"), ("r", "d2"),
             ("r", "p1"), ("x", "v0"), ("r", "v0"), ("r", "p2"),
             ("r", "p3")]
assert sum(w for _, w, _ in EM_CHUNKS) == V

_CACHED = {}

# exposed for test harnesses: the BassKernelResults of the last run
LAST_RESULTS = None


def _build_bass():
    nc = Bacc(trn_type="TRN2")

    em_s = nc.dram_tensor("em_s", [HP, V], FP8, kind="ExternalInput")
    tm_s = nc.dram_tensor("tm_s", [HP, H], BF16, kind="ExternalInput")

    rs_out = nc.dram_tensor("rs_out", [HP, len(EM_CHUNKS)], F32,
                            kind="ExternalOutput")
    tme_out = nc.dram_tensor("tme_out", [HP, H], BF16, kind="ExternalOutput")

    kinds = {n: k for n, _, k in EM_CHUNKS}
    widths = {n: w for n, w, _ in EM_CHUNKS}
    col = {n: i for i, (n, _, _) in enumerate(EM_CHUNKS)}

    with tile.TileContext(nc) as tc:
        with (
            tc.tile_pool(name="const", bufs=1) as const,
            tc.tile_pool(name="ld", bufs=8) as ld,
            tc.tile_pool(name="ip", bufs=5) as ip,
            tc.tile_pool(name="sc", bufs=5) as sc,
        ):
            rs_parts = const.tile([HP, len(EM_CHUNKS)], F32)
            tm_t = const.tile([HP, H], BF16)
            tm_e = const.tile([HP, H], BF16)

            em_tiles, st_tiles, it_tiles = {}, {}, {}
            off = 0
            for n in DMA_ORDER:
                if n == "tm":
                    nc.sync.dma_start(tm_t, tm_s[:, :])
                    continue
                w = widths[n]
                em_t = ld.tile([HP, w], FP8, name=f"em_{n}", tag="em")
                nc.sync.dma_start(em_t, em_s[:, off:off + w])
                em_tiles[n] = em_t
                off += w
            assert off == V

            for n in ACT_ORDER:
                if n == "tm":
                    nc.scalar.activation(tm_e, tm_t, AF.Exp)
                    continue
                w = widths[n]
                if kinds[n] == "acc":
                    nc.scalar.activation(
                        em_tiles[n], em_tiles[n], AF.Exp,
                        accum_out=rs_parts[:, col[n]:col[n] + 1],
                    )
                else:
                    st = sc.tile([HP, w], BF16, name=f"st_{n}", tag="st")
                    st_tiles[n] = st
                    nc.scalar.activation(st, em_tiles[n], AF.Exp)

            for n in POOL_ORDER:
                it = ip.tile([HP, widths[n]], I16, name=f"it_{n}", tag="it")
                it_tiles[n] = it
                nc.gpsimd.tensor_scalar(
                    it, em_tiles[n], SCH_A16, SCH_B16,
                    mybir.AluOpType.mult, mybir.AluOpType.add,
                )

            # VectorE queue: row-sums via tensor_scalar+accum (4x packed
            # mode; in-place writes keep SBUF pressure down) interleaved
            # with its own fast-exp on the 'vex' chunk
            for op, n in DVE_ORDER:
                if op == "x":
                    it = ip.tile([HP, widths[n]], I16, name=f"it_{n}",
                                 tag="it")
                    it_tiles[n] = it
                    nc.vector.tensor_scalar(
                        it, em_tiles[n], SCH_A16, SCH_B16,
                        mybir.AluOpType.mult, mybir.AluOpType.add,
                    )
                    continue
                src = (st_tiles[n] if kinds[n] == "dve"
                       else it_tiles[n].bitcast(BF16))
                nc.vector.tensor_scalar(
                    src, src, 1.0, 0.0,
                    mybir.AluOpType.mult, mybir.AluOpType.add,
                    accum_out=rs_parts[:, col[n]:col[n] + 1],
                )

            # ship unnormalized exp(tm) rows; host normalizes + column-sums
            nc.sync.dma_start(tme_out[:, :], tm_e)
            nc.sync.dma_start(rs_out[:, :], rs_parts)

    nc.finalize()
    return nc


def _logsumexp(x, axis):
    m = np.max(x, axis=axis, keepdims=True)
    return np.squeeze(m, axis) + np.log(np.sum(np.exp(x - m), axis=axis))


def kernel(input_ids, do_em, em, tm, p):
    global LAST_RESULTS

    ids = np.asarray(input_ids).astype(np.int64)
    em = np.ascontiguousarray(np.asarray(em, dtype=np.float32))
    tm = np.ascontiguousarray(np.asarray(tm, dtype=np.float32))
    p64 = np.asarray(p, dtype=np.float64)

    if "nc" not in _CACHED:
        _CACHED["nc"] = _build_bass()
    nc = _CACHED["nc"]

    em_q = em.astype(mybir.dt.np(FP8))
    tm_b = tm.astype(mybir.dt.np(BF16))

    in_maps = [
        {
            "em_s": em_q[k * HP:(k + 1) * HP],
            "tm_s": tm_b[k * HP:(k + 1) * HP],
        }
        for k in range(N_CORES)
    ]
    res = run_bass_kernel_spmd(nc, in_maps, core_ids=list(range(N_CORES)))
    LAST_RESULTS = res

    # de-bias the Schraudolph partial sums, then total per row
    pool_cols = np.array(
        [i for i, (_, _, k) in enumerate(EM_CHUNKS) if k in ("pool", "vex")])
    exact_cols = np.array(
        [i for i, (_, _, k) in enumerate(EM_CHUNKS) if k in ("dve", "acc")])
    rowsum_parts = []
    for k in range(N_CORES):
        rs = res.results[k]["rs_out"].astype(np.float64)      # (HP, nch)
        rowsum_parts.append(
            rs[:, exact_cols].sum(axis=1)
            + rs[:, pool_cols].sum(axis=1) / POOL_CAL
        )
    rowsum = np.concatenate(rowsum_parts)                     # (H,)

    tm_colsum = np.zeros(H, dtype=np.float64)
    for k in range(N_CORES):
        tme = res.results[k]["tme_out"].astype(np.float64)    # (HP, H)
        tm_colsum += (tme / tme.sum(axis=1, keepdims=True)).sum(axis=0)

    # token histogram + small gather-GEMM on host
    counts = np.zeros((V, B), dtype=np.float32)
    for b in range(B):
        np.add.at(counts[:, b], ids[b], 1.0)
    G = (em @ counts).astype(np.float64)                      # (H, B)

    row_lse = np.log(rowsum)
    c = np.log(tm_colsum)
    p_ls = p64 - _logsumexp(p64[None, :], 1)[0]

    alpha = p_ls[None, :] + (S - 1) * c[None, :] + G.T - S * row_lse[None, :]
    ll = _logsumexp(alpha, 1)                                 # (B,)
    return np.float32(-np.mean(ll))
